# revision 23
# baseline (speedup 1.0000x reference)
"""Causal self-attention (GQA + RoPE) TP-sharded over 8 trn2 NeuronCores.

Sharding: core c owns Q heads {2c, 2c+1} and KV head c//2 (GQA rep=4 means
both Q heads map to the same KV head). Each core computes its head-shard of
q/k/v projections + rotary + causal attention + a partial o_proj against its
256-column shard of Wo. The host sums the 8 partial outputs.

All matmuls in bf16 (fp8 measured 5% output error: y's relative error equals
the scores' absolute error, so 8-bit mantissa is required in the q/k path).

Key structure:
  - V projected directly into natural [t, d] layout (x chunk as stationary),
    no PE transposes.
  - scores kept transposed [tk, tq]; softmax denominator via a Pool-engine
    accumulation of exp'd P chunks followed by ONE ones-matmul per (h, j)
    (instead of a ones-matmul per chunk). No max subtraction (scores O(1)).
  - normalization reciprocal broadcast across partitions via a DRAM
    round-trip DMA; o_proj for tile j is deferred into attention of tile
    j+1 so the round trip latency is hidden.
  - projection / o_proj matmuls are interleaved instruction-by-instruction
    into the attention chunk loops ("background pump") so the PE never
    stalls on the ACT exp chain.
"""

import sys

try:
    import concourse.bass as bass  # noqa: F401
except ImportError:
    sys.path.insert(0, "/opt/trn_rl_repo")

import math
from collections import deque
from contextlib import ExitStack

import ml_dtypes
import numpy as np

import concourse.bass as bass
import concourse.mybir as mybir
import concourse.tile as tile
from concourse import bacc
from concourse.bass_utils import run_bass_kernel_spmd

F32 = mybir.dt.float32
F32R = mybir.dt.float32r
BF16 = mybir.dt.bfloat16
NP_BF16 = ml_dtypes.bfloat16

B, T, C = 2, 2048, 2048
BT = B * T
N_HEAD, N_KV_HEAD, HD = 16, 4, 128
ROTARY_BASE = 10000
N_CORES = 8
QSH = 2 * HD  # q output dims per core (2 heads)
SCALE = 1.0 / math.sqrt(HD)

TT = 512  # t-tile (moving-operand free size)
NT = T // TT  # t tiles per batch (4)
KC = C // 128  # contraction chunks for projections (16)

POOL_ROWSUM = True  # accumulate P on Pool engine, one ones-matmul per (h,j)


def _sin_cos_np():
    # mirror reference._sin_cos bit-for-bit (float32 throughout)
    pos = np.arange(T, dtype=np.float32)
    dim = np.arange(HD // 2, dtype=np.float32)
    freq = (np.float32(ROTARY_BASE) ** (dim / np.float32(HD / 2))).astype(np.float32)
    freq = np.concatenate([freq, freq])
    angles = pos[:, None] / freq[None, :]
    return np.sin(angles).astype(np.float32), np.cos(angles).astype(np.float32)


def build_kernel():
    nc = bacc.Bacc()
    xT = nc.dram_tensor("xT", [C, BT], BF16, kind="ExternalInput")
    wq = nc.dram_tensor("wq", [128, KC * QSH], BF16, kind="ExternalInput")
    wk = nc.dram_tensor("wk", [128, KC * HD], BF16, kind="ExternalInput")
    wv = nc.dram_tensor("wv", [128, KC * HD], BF16, kind="ExternalInput")
    wo = nc.dram_tensor("wo", [128, 2 * C], BF16, kind="ExternalInput")
    cosd = nc.dram_tensor("cosd", [HD, T], F32, kind="ExternalInput")
    sind = nc.dram_tensor("sind", [HD, T], F32, kind="ExternalInput")  # rot+signed
    trid = nc.dram_tensor("trid", [128, 128], BF16, kind="ExternalInput")
    onesd = nc.dram_tensor("onesd", [128, 1], F32R, kind="ExternalInput")
    onesbd = nc.dram_tensor("onesbd", [128, 1], BF16, kind="ExternalInput")
    out = nc.dram_tensor("out", [BT, C], BF16, kind="ExternalOutput")

    with ExitStack() as ctx:
        tc = ctx.enter_context(tile.TileContext(nc))
        consts = ctx.enter_context(tc.tile_pool(name="consts", bufs=1))
        xpool = ctx.enter_context(tc.tile_pool(name="xc", bufs=32))
        qkpool = ctx.enter_context(tc.tile_pool(name="qk", bufs=10))
        kpool = ctx.enter_context(tc.tile_pool(name="kT", bufs=5))
        vpool = ctx.enter_context(tc.tile_pool(name="vnat", bufs=5))
        tmppool = ctx.enter_context(tc.tile_pool(name="ropetmp", bufs=4))
        ppool = ctx.enter_context(tc.tile_pool(name="pT", bufs=5))
        accpool = ctx.enter_context(tc.tile_pool(name="acc", bufs=2))
        ytpool = ctx.enter_context(tc.tile_pool(name="yT", bufs=10))
        rcpool = ctx.enter_context(tc.tile_pool(name="rcp", bufs=3))
        rbcpool = ctx.enter_context(tc.tile_pool(name="rbc", bufs=4))
        outpool = ctx.enter_context(tc.tile_pool(name="osb", bufs=4))
        drampool = ctx.enter_context(
            tc.tile_pool(name="dscratch", bufs=4, space="DRAM")
        )
        # Dedicated PSUM pools per role: the background-pump emission
        # interleaves phases, so a single rotating pool could hand a bank to
        # a pumped matmul while an earlier tile still has pending
        # accumulation writes later in the PE queue. Per-role pools keep
        # allocation order == usage order within each bank set.
        ps_proj = ctx.enter_context(tc.tile_pool(name="psproj", bufs=2, space="PSUM"))
        ps_s = ctx.enter_context(tc.tile_pool(name="pss", bufs=2, space="PSUM"))
        ps_y = ctx.enter_context(tc.tile_pool(name="psy", bufs=1, space="PSUM"))
        ps_r = ctx.enter_context(tc.tile_pool(name="psr", bufs=1, space="PSUM"))
        ps_op = ctx.enter_context(tc.tile_pool(name="psop", bufs=2, space="PSUM"))

        def pstile(pool, shape, dtype, name):
            return pool.tile(shape, dtype, tag="ps", name=name)

        # resident weights on the ACT queue (sync queue serves x tiles)
        wq_sb = consts.tile([128, KC, QSH], BF16)
        nc.scalar.dma_start(out=wq_sb, in_=wq.ap())
        wk_sb = consts.tile([128, KC, HD], BF16)
        nc.scalar.dma_start(out=wk_sb, in_=wk.ap())
        wv_sb = consts.tile([128, KC, HD], BF16)
        nc.scalar.dma_start(out=wv_sb, in_=wv.ap())

        wo_sb = consts.tile([128, 2, C], BF16)
        cos_sb = consts.tile([HD, T], F32)
        sin_sb = consts.tile([HD, T], F32)
        tri_sb = consts.tile([128, 128], BF16)
        ones_sb = consts.tile([128, 1], F32R)
        onesb_sb = consts.tile([128, 1], BF16)

        def load_late_consts():
            nc.scalar.dma_start(out=cos_sb, in_=cosd.ap())
            nc.scalar.dma_start(out=sin_sb, in_=sind.ap())
            nc.scalar.dma_start(out=ones_sb, in_=onesd.ap())
            nc.scalar.dma_start(out=onesb_sb, in_=onesbd.ap())
            nc.scalar.dma_start(out=tri_sb, in_=trid.ap())
            nc.scalar.dma_start(out=wo_sb, in_=wo.ap())

        xT_ap = xT.ap()
        out_ap = out.ap()

        def rope_evac(dst, pj, tpos):
            """dst[bf16] = pj*cos + rotate_half(pj)*sin, psum -> sbuf.

            sind rows are pre-rotated by 64 and sign-folded on the host.
            tmp/tmp2 stay fp32 so no op mixes input dtypes.
            """
            cs = cos_sb[:, tpos : tpos + TT]
            sn = sin_sb[:, tpos : tpos + TT]
            tmp = tmppool.tile([128, TT], F32, tag="tmp", name="rt1")
            tmp2 = tmppool.tile([128, TT], F32, tag="tmp", name="rt2")
            nc.vector.tensor_mul(tmp[0:64], pj[64:128], sn[64:128])
            nc.vector.tensor_mul(tmp[64:128], pj[0:64], sn[0:64])
            nc.vector.tensor_mul(tmp2, pj, cs)  # last psum read: frees the bank
            nc.vector.tensor_add(dst, tmp2, tmp)

        # ---------------- emission helpers ----------------
        state = {}  # per-batch tile handles

        def proj_thunks(b, jt):
            """Thunk list for projections of (b, jt): x DMA, Q/K matmuls +
            rope, V natural matmuls + evac."""
            tcol = b * T + jt * TT
            tpos = jt * TT
            qT, kT, v_sb = state[b]["qT"], state[b]["kT"], state[b]["v"]
            th = []
            xc = [None] * KC
            pq = [None, None]
            pk = [None]
            pv = [None]

            def dma_x():
                for kc in range(KC):
                    xc[kc] = xpool.tile(
                        [128, TT], BF16, tag="xc", name=f"xc_{b}_{jt}_{kc}"
                    )
                    nc.sync.dma_start(
                        out=xc[kc],
                        in_=xT_ap[128 * kc : 128 * kc + 128, tcol : tcol + TT],
                    )

            th.append(dma_x)

            def alloc_pq():
                pq[0] = pstile(ps_proj, [128, TT], F32, f"pq_{b}_{jt}_0")
                pq[1] = pstile(ps_proj, [128, TT], F32, f"pq_{b}_{jt}_1")

            th.append(alloc_pq)
            for kc in range(KC):
                for h in range(2):
                    def mm_q(kc=kc, h=h):
                        nc.tensor.matmul(
                            pq[h],
                            wq_sb[:, kc, 128 * h : 128 * h + 128],
                            xc[kc],
                            start=(kc == 0),
                            stop=(kc == KC - 1),
                        )
                    th.append(mm_q)
            th.append(lambda: rope_evac(qT[0][jt], pq[0], tpos))
            th.append(lambda: rope_evac(qT[1][jt], pq[1], tpos))

            def alloc_pk():
                pk[0] = pstile(ps_proj, [128, TT], F32, f"pk_{b}_{jt}")

            th.append(alloc_pk)
            for kc in range(KC):
                def mm_k(kc=kc):
                    nc.tensor.matmul(
                        pk[0],
                        wk_sb[:, kc, :],
                        xc[kc],
                        start=(kc == 0),
                        stop=(kc == KC - 1),
                    )
                th.append(mm_k)
            th.append(lambda: rope_evac(kT[jt], pk[0], tpos))

            def alloc_pv():
                pv[0] = pstile(ps_proj, [128, 4, HD], F32, f"pv_{b}_{jt}")

            th.append(alloc_pv)
            for tb in range(4):
                for kc in range(KC):
                    def mm_v(tb=tb, kc=kc):
                        nc.tensor.matmul(
                            pv[0][:, tb, :],
                            xc[kc][:, 128 * tb : 128 * tb + 128],
                            wv_sb[:, kc, :],
                            start=(kc == 0),
                            stop=(kc == KC - 1),
                        )
                    th.append(mm_v)
            th.append(lambda: nc.scalar.copy(state[b]["v"][jt], pv[0]))
            return th

        def oproj_thunks(b, j):
            """Thunk list for o_proj of row tile j (deferred one phase so the
            reciprocal broadcast DMA has landed)."""
            yT = state[b]["yT"]
            rbcs = state[b]["rbc"]
            th = []
            for h in range(2):
                def ymul(h=h):
                    nc.vector.tensor_mul(yT[h][j], yT[h][j], rbcs[h][j])
                th.append(ymul)
            for t4 in range(4):
                row = b * T + 512 * j + 128 * t4
                osb = [None]

                def alloc_osb(t4=t4, osb=osb):
                    osb[0] = outpool.tile(
                        [128, C], BF16, tag="osb", name=f"osb_{b}_{j}_{t4}"
                    )

                th.append(alloc_osb)
                for n in range(C // TT):
                    op = [None]

                    def mm_o(t4=t4, n=n, op=op, osb=osb):
                        op[0] = pstile(ps_op, [128, TT], F32, f"op_{b}_{j}_{t4}_{n}")
                        nc.tensor.matmul(
                            op[0],
                            yT[0][j][:, 128 * t4 : 128 * t4 + 128],
                            wo_sb[:, 0, TT * n : TT * n + TT],
                            start=True,
                            stop=False,
                        )
                        nc.tensor.matmul(
                            op[0],
                            yT[1][j][:, 128 * t4 : 128 * t4 + 128],
                            wo_sb[:, 1, TT * n : TT * n + TT],
                            start=False,
                            stop=True,
                        )

                    th.append(mm_o)

                    def evac_o(n=n, op=op, osb=osb):
                        osl = osb[0][:, TT * n : TT * n + TT]
                        if n < 2:
                            nc.scalar.copy(osl, op[0])
                        else:
                            nc.vector.tensor_copy(osl, op[0])

                    th.append(evac_o)

                def dma_o(row=row, osb=osb):
                    nc.scalar.dma_start(
                        out=out_ap[row : row + 128, :], in_=osb[0]
                    )

                th.append(dma_o)
            return th

        bg = deque()

        def pump(n):
            for _ in range(n):
                if not bg:
                    return
                bg.popleft()()

        def drain():
            while bg:
                bg.popleft()()

        def attention(b, j):
            qT, kT, v_sb, yT, rbcs = (
                state[b]["qT"],
                state[b]["kT"],
                state[b]["v"],
                state[b]["yT"],
                state[b]["rbc"],
            )
            if j == 0:
                chunks = [(m, 128 * m) for m in (0, 1, 2, 3)]
            else:
                chunks = [(0, 0)]
                chunks += [(4 * j + m, 128 * m) for m in (0, 1, 2, 3)]
                chunks += [(c, 0) for c in range(1, 4 * j)]
            nch = len(chunks)
            # thunks to pump per chunk so bg drains over this (h, j) phase
            per = max(1, (len(bg) + 2 * nch - 1) // (2 * nch))
            for h in range(2):
                yp = pstile(ps_y, [128, TT], F32, f"yp_{b}_{h}_{j}")
                acc = (
                    accpool.tile(
                        [128, TT], F32R, tag="acc", name=f"acc_{b}_{h}_{j}"
                    )
                    if POOL_ROWSUM
                    else None
                )
                sT = [None] * nch
                pT = [None] * nch

                def emit_scores(i):
                    cch, off = chunks[i]
                    sT[i] = pstile(ps_s, [128, TT - off], F32, f"sT_{b}_{h}_{j}_{i}")
                    nc.tensor.matmul(
                        sT[i],
                        kT[cch // 4][:, 128 * (cch % 4) : 128 * (cch % 4) + 128],
                        qT[h][j][:, off:],
                        start=True,
                        stop=True,
                    )

                if not POOL_ROWSUM:
                    rp = pstile(ps_r, [1, TT], F32, f"rp_{b}_{h}_{j}")
                emit_scores(0)
                for i in range(nch):
                    cch, off = chunks[i]
                    pT[i] = ppool.tile(
                        [128, TT - off], BF16, tag="p", name=f"pT_{b}_{h}_{j}_{i}"
                    )
                    nc.scalar.activation(
                        out=pT[i],
                        in_=sT[i],
                        func=mybir.ActivationFunctionType.Exp,
                        scale=SCALE,
                    )
                    if cch >= 4 * j:  # diagonal block: causal triangle
                        nc.vector.tensor_mul(pT[i][:, 0:128], pT[i][:, 0:128], tri_sb)
                    if i + 1 < nch:
                        emit_scores(i + 1)
                    pump(per)
                    nc.tensor.matmul(
                        yp[:, off:],
                        v_sb[cch // 4][:, cch % 4, :],
                        pT[i],
                        start=(i == 0),
                        stop=(i == nch - 1),
                    )
                    if POOL_ROWSUM:
                        # Pool-engine softmax-denominator accumulation
                        if i == 0:
                            nc.gpsimd.tensor_copy(acc, pT[i])
                        else:
                            nc.gpsimd.tensor_add(acc[:, off:], acc[:, off:], pT[i])
                    else:
                        nc.tensor.matmul(
                            rp[:, off:],
                            onesb_sb,
                            pT[i],
                            start=(i == 0),
                            stop=(i == nch - 1),
                        )
                if POOL_ROWSUM:
                    # one rowsum matmul per (h, j) from the accumulated P
                    rp = pstile(ps_r, [1, TT], F32, f"rp_{b}_{h}_{j}")
                    nc.tensor.matmul(rp, ones_sb, acc, start=True, stop=True)
                rcp = rcpool.tile([1, TT], BF16, tag="rcp", name=f"rcp_{b}_{h}_{j}")
                with nc.allow_low_precision(reason="softmax denom bf16"):
                    nc.vector.reciprocal(rcp, rp)
                nc.scalar.copy(yT[h][j], yp)  # frees the PV bank
                rdr = drampool.tile([1, TT], BF16, tag="rdr", name=f"rdr_{b}_{h}_{j}")
                nc.sync.dma_start(out=rdr, in_=rcp)
                rbc = rbcpool.tile([128, TT], BF16, tag="rbc", name=f"rbc_{b}_{h}_{j}")
                nc.sync.dma_start(
                    out=rbc,
                    in_=bass.AP(
                        tensor=rdr.tensor,
                        offset=rdr.offset,
                        ap=[[0, 128], rdr.ap[-1]],
                    ),
                )
                rbcs[h][j] = rbc

        for b in range(B):
            state[b] = {
                "qT": [
                    [
                        qkpool.tile([128, TT], BF16, tag="qT", name=f"qT_{b}_{h}_{j}")
                        for j in range(NT)
                    ]
                    for h in range(2)
                ],
                "kT": [
                    kpool.tile([128, TT], BF16, tag="kT", name=f"kT_{b}_{j}")
                    for j in range(NT)
                ],
                "v": [
                    vpool.tile([128, 4, HD], BF16, tag="v", name=f"v_{b}_{j}")
                    for j in range(NT)
                ],
                "yT": [
                    [
                        ytpool.tile([128, TT], BF16, tag="yT", name=f"yT_{b}_{h}_{j}")
                        for j in range(NT)
                    ]
                    for h in range(2)
                ],
                "rbc": [[None] * NT for _ in range(2)],
            }

        # startup: first projection tile inline. Emit the const DMAs right
        # after the x-tile DMAs (sync queue) so cos/sin land before the
        # first rope reads them — they go on the scalar queue concurrently.
        _startup = proj_thunks(0, 0)
        _startup[0]()  # x DMAs
        load_late_consts()
        for t in _startup[1:]:
            t()
        for b in range(B):
            for j in range(NT):
                if j + 1 < NT:
                    bg.extend(proj_thunks(b, j + 1))
                elif b + 1 < B:
                    bg.extend(proj_thunks(b + 1, 0))
                if j >= 1:
                    bg.extend(oproj_thunks(b, j - 1))
                elif b >= 1:
                    bg.extend(oproj_thunks(b - 1, NT - 1))
                attention(b, j)
                drain()
        # o_proj of the final tile has nothing left to hide behind
        for t in oproj_thunks(B - 1, NT - 1):
            t()

    nc.finalize()
    return nc


_NC_CACHE = None
TRACE = False
LAST_RESULTS = None


def _get_nc():
    global _NC_CACHE
    if _NC_CACHE is None:
        _NC_CACHE = build_kernel()
    return _NC_CACHE


def kernel(x, Wq, Wk, Wv, Wo):
    x = np.asarray(x, dtype=np.float32)
    Wq = np.asarray(Wq, dtype=np.float32)
    Wk = np.asarray(Wk, dtype=np.float32)
    Wv = np.asarray(Wv, dtype=np.float32)
    Wo = np.asarray(Wo, dtype=np.float32)

    xT = np.ascontiguousarray(x.reshape(BT, C).T).astype(NP_BF16)
    sin_, cos_ = _sin_cos_np()  # [T, 128]
    cosd = np.ascontiguousarray(cos_.T)
    sinT = np.ascontiguousarray(sin_.T)
    sind = np.empty_like(sinT)
    sind[64:128] = -sinT[0:64]
    sind[0:64] = sinT[64:128]
    trid = np.triu(np.ones((128, 128), dtype=np.float32)).astype(NP_BF16)
    onesd = np.ones((128, 1), dtype=np.float32)
    onesbd = np.ones((128, 1), dtype=NP_BF16)

    core_ids = list(range(N_CORES))
    in_maps = []
    for c in core_ids:
        g = c // 2
        wqT = Wq[QSH * c : QSH * (c + 1)].T  # [C, 256]
        wq_h = np.ascontiguousarray(
            wqT.reshape(KC, 128, QSH).transpose(1, 0, 2)
        ).reshape(128, KC * QSH).astype(NP_BF16)
        wkT = Wk[HD * g : HD * (g + 1)].T
        wk_h = np.ascontiguousarray(
            wkT.reshape(KC, 128, HD).transpose(1, 0, 2)
        ).reshape(128, KC * HD).astype(NP_BF16)
        wvT = Wv[HD * g : HD * (g + 1)].T
        wv_h = np.ascontiguousarray(
            wvT.reshape(KC, 128, HD).transpose(1, 0, 2)
        ).reshape(128, KC * HD).astype(NP_BF16)
        woT = Wo[:, QSH * c : QSH * (c + 1)].T  # [256, C]
        wo_h = np.ascontiguousarray(
            woT.reshape(2, 128, C).transpose(1, 0, 2)
        ).reshape(128, 2 * C).astype(NP_BF16)
        in_maps.append(
            {
                "xT": xT,
                "wq": wq_h,
                "wk": wk_h,
                "wv": wv_h,
                "wo": wo_h,
                "cosd": cosd,
                "sind": sind,
                "trid": trid,
                "onesd": onesd,
                "onesbd": onesbd,
            }
        )
    global LAST_RESULTS
    res = run_bass_kernel_spmd(_get_nc(), in_maps, core_ids, trace=TRACE)
    LAST_RESULTS = res
    total = res.results[0]["out"].astype(np.float32)
    for c in core_ids[1:]:
        total = total + res.results[c]["out"].astype(np.float32)
    return total.reshape(B, T, C)


# revision 30
# speedup vs baseline: 1.1405x; 1.1405x over previous
"""Causal self-attention (GQA + RoPE) TP-sharded over 8 trn2 NeuronCores.

Sharding: core c owns Q heads {2c, 2c+1} and KV head c//2 (GQA rep=4 means
both Q heads map to the same KV head). Each core computes its head-shard of
q/k/v projections + rotary + causal attention + a partial o_proj against its
256-column shard of Wo. The host sums the 8 partial outputs.

All matmuls in bf16 (fp8 measured 5% output error: y's relative error equals
the scores' absolute error, so 8-bit mantissa is required in the q/k path).

Key structure:
  - V projected directly into natural [t, d] layout (x chunk as stationary),
    no PE transposes.
  - scores kept transposed [tk, tq]; softmax denominator via a Pool-engine
    accumulation of exp'd P chunks followed by ONE ones-matmul per (h, j)
    (instead of a ones-matmul per chunk). No max subtraction (scores O(1)).
  - normalization reciprocal broadcast across partitions via a DRAM
    round-trip DMA; o_proj for tile j is deferred into attention of tile
    j+1 so the round trip latency is hidden.
  - projection / o_proj matmuls are interleaved instruction-by-instruction
    into the attention chunk loops ("background pump") so the PE never
    stalls on the ACT exp chain.
"""

import sys

try:
    import concourse.bass as bass  # noqa: F401
except ImportError:
    sys.path.insert(0, "/opt/trn_rl_repo")

import math
from collections import deque
from contextlib import ExitStack

import ml_dtypes
import numpy as np

import concourse.bass as bass
import concourse.mybir as mybir
import concourse.tile as tile
from concourse import bacc
from concourse.bass_utils import run_bass_kernel_spmd

F32 = mybir.dt.float32
F32R = mybir.dt.float32r
BF16 = mybir.dt.bfloat16
NP_BF16 = ml_dtypes.bfloat16

B, T, C = 2, 2048, 2048
BT = B * T
N_HEAD, N_KV_HEAD, HD = 16, 4, 128
ROTARY_BASE = 10000
N_CORES = 8
QSH = 2 * HD  # q output dims per core (2 heads)
SCALE = 1.0 / math.sqrt(HD)

TT = 512  # t-tile (moving-operand free size)
NT = T // TT  # t tiles per batch (4)
KC = C // 128  # contraction chunks for projections (16)

POOL_ROWSUM = False  # accumulate P on Pool engine, one ones-matmul per (h,j)


def _sin_cos_np():
    # mirror reference._sin_cos bit-for-bit (float32 throughout)
    pos = np.arange(T, dtype=np.float32)
    dim = np.arange(HD // 2, dtype=np.float32)
    freq = (np.float32(ROTARY_BASE) ** (dim / np.float32(HD / 2))).astype(np.float32)
    freq = np.concatenate([freq, freq])
    angles = pos[:, None] / freq[None, :]
    return np.sin(angles).astype(np.float32), np.cos(angles).astype(np.float32)


def build_kernel():
    nc = bacc.Bacc()
    xT = nc.dram_tensor("xT", [C, BT], BF16, kind="ExternalInput")
    wq = nc.dram_tensor("wq", [128, KC * QSH], BF16, kind="ExternalInput")
    wk = nc.dram_tensor("wk", [128, KC * HD], BF16, kind="ExternalInput")
    wv = nc.dram_tensor("wv", [128, KC * HD], BF16, kind="ExternalInput")
    wo = nc.dram_tensor("wo", [128, 2 * C], BF16, kind="ExternalInput")
    cosd = nc.dram_tensor("cosd", [HD, T], F32, kind="ExternalInput")
    sind = nc.dram_tensor("sind", [HD, T], F32, kind="ExternalInput")  # rot+signed
    trid = nc.dram_tensor("trid", [128, 128], BF16, kind="ExternalInput")
    onesd = nc.dram_tensor("onesd", [128, 1], F32R, kind="ExternalInput")
    onesbd = nc.dram_tensor("onesbd", [128, 1], BF16, kind="ExternalInput")
    onesrd = nc.dram_tensor("onesrd", [1, 128], F32R, kind="ExternalInput")
    out = nc.dram_tensor("out", [BT, C], BF16, kind="ExternalOutput")

    with ExitStack() as ctx:
        tc = ctx.enter_context(tile.TileContext(nc))
        consts = ctx.enter_context(tc.tile_pool(name="consts", bufs=1))
        xpool = ctx.enter_context(tc.tile_pool(name="xc", bufs=32))
        qkpool = ctx.enter_context(tc.tile_pool(name="qk", bufs=10))
        kpool = ctx.enter_context(tc.tile_pool(name="kT", bufs=5))
        vpool = ctx.enter_context(tc.tile_pool(name="vnat", bufs=5))
        tmppool = ctx.enter_context(tc.tile_pool(name="ropetmp", bufs=4))
        ppool = ctx.enter_context(tc.tile_pool(name="pT", bufs=5))
        accpool = ctx.enter_context(tc.tile_pool(name="acc", bufs=2))
        ytpool = ctx.enter_context(tc.tile_pool(name="yT", bufs=10))
        rcpool = ctx.enter_context(tc.tile_pool(name="rcp", bufs=3))
        rbcpool = ctx.enter_context(tc.tile_pool(name="rbc", bufs=4))
        outpool = ctx.enter_context(tc.tile_pool(name="osb", bufs=4))
        drampool = ctx.enter_context(
            tc.tile_pool(name="dscratch", bufs=4, space="DRAM")
        )
        # Dedicated PSUM pools per role: the background-pump emission
        # interleaves phases, so a single rotating pool could hand a bank to
        # a pumped matmul while an earlier tile still has pending
        # accumulation writes later in the PE queue. Per-role pools keep
        # allocation order == usage order within each bank set.
        ps_proj = ctx.enter_context(tc.tile_pool(name="psproj", bufs=2, space="PSUM"))
        ps_s = ctx.enter_context(tc.tile_pool(name="pss", bufs=2, space="PSUM"))
        ps_y = ctx.enter_context(tc.tile_pool(name="psy", bufs=1, space="PSUM"))
        ps_r = ctx.enter_context(tc.tile_pool(name="psr", bufs=1, space="PSUM"))
        ps_op = ctx.enter_context(tc.tile_pool(name="psop", bufs=2, space="PSUM"))

        def pstile(pool, shape, dtype, name):
            return pool.tile(shape, dtype, tag="ps", name=name)

        # resident weights on the ACT queue (sync queue serves x tiles)
        wq_sb = consts.tile([128, KC, QSH], BF16)
        nc.scalar.dma_start(out=wq_sb, in_=wq.ap())
        wk_sb = consts.tile([128, KC, HD], BF16)
        nc.scalar.dma_start(out=wk_sb, in_=wk.ap())
        wv_sb = consts.tile([128, KC, HD], BF16)
        nc.scalar.dma_start(out=wv_sb, in_=wv.ap())

        wo_sb = consts.tile([128, 2, C], BF16)
        cos_sb = consts.tile([HD, T], F32)
        sin_sb = consts.tile([HD, T], F32)
        tri_sb = consts.tile([128, 128], BF16)
        ones_sb = consts.tile([128, 1], F32R)
        onesb_sb = consts.tile([128, 1], BF16)
        onesr_sb = consts.tile([1, 128], F32R)

        def load_late_consts():
            nc.scalar.dma_start(out=cos_sb, in_=cosd.ap())
            nc.scalar.dma_start(out=sin_sb, in_=sind.ap())
            nc.scalar.dma_start(out=ones_sb, in_=onesd.ap())
            nc.scalar.dma_start(out=onesb_sb, in_=onesbd.ap())
            nc.scalar.dma_start(out=onesr_sb, in_=onesrd.ap())
            nc.scalar.dma_start(out=tri_sb, in_=trid.ap())
            nc.scalar.dma_start(out=wo_sb, in_=wo.ap())

        xT_ap = xT.ap()
        out_ap = out.ap()

        def rope_evac(dst, pj, tpos):
            """dst[bf16] = pj*cos + rotate_half(pj)*sin, psum -> sbuf.

            sind rows are pre-rotated by 64 and sign-folded on the host.
            tmp/tmp2 stay fp32 so no op mixes input dtypes.
            """
            cs = cos_sb[:, tpos : tpos + TT]
            sn = sin_sb[:, tpos : tpos + TT]
            tmp = tmppool.tile([128, TT], F32, tag="tmp", name="rt1")
            tmp2 = tmppool.tile([128, TT], F32, tag="tmp", name="rt2")
            nc.vector.tensor_mul(tmp[0:64], pj[64:128], sn[64:128])
            nc.vector.tensor_mul(tmp[64:128], pj[0:64], sn[0:64])
            nc.vector.tensor_mul(tmp2, pj, cs)  # last psum read: frees the bank
            nc.vector.tensor_add(dst, tmp2, tmp)

        # ---------------- emission helpers ----------------
        state = {}  # per-batch tile handles

        def proj_thunks(b, jt):
            """Thunk list for projections of (b, jt): x DMA, Q/K matmuls +
            rope, V natural matmuls + evac."""
            tcol = b * T + jt * TT
            tpos = jt * TT
            qT, kT, v_sb = state[b]["qT"], state[b]["kT"], state[b]["v"]
            th = []
            xc = [None] * KC
            pq = [None, None]
            pk = [None]
            pv = [None]

            def dma_x():
                for kc in range(KC):
                    xc[kc] = xpool.tile(
                        [128, TT], BF16, tag="xc", name=f"xc_{b}_{jt}_{kc}"
                    )
                    eng = nc.sync if kc % 2 == 0 else nc.scalar
                    eng.dma_start(
                        out=xc[kc],
                        in_=xT_ap[128 * kc : 128 * kc + 128, tcol : tcol + TT],
                    )

            th.append(dma_x)

            def alloc_pq():
                pq[0] = pstile(ps_proj, [128, TT], F32, f"pq_{b}_{jt}_0")
                pq[1] = pstile(ps_proj, [128, TT], F32, f"pq_{b}_{jt}_1")

            th.append(alloc_pq)
            for kc in range(KC):
                for h in range(2):
                    def mm_q(kc=kc, h=h):
                        nc.tensor.matmul(
                            pq[h],
                            wq_sb[:, kc, 128 * h : 128 * h + 128],
                            xc[kc],
                            start=(kc == 0),
                            stop=(kc == KC - 1),
                        )
                    th.append(mm_q)
            th.append(lambda: rope_evac(qT[0][jt], pq[0], tpos))
            th.append(lambda: rope_evac(qT[1][jt], pq[1], tpos))

            def alloc_pk():
                pk[0] = pstile(ps_proj, [128, TT], F32, f"pk_{b}_{jt}")

            th.append(alloc_pk)
            for kc in range(KC):
                def mm_k(kc=kc):
                    nc.tensor.matmul(
                        pk[0],
                        wk_sb[:, kc, :],
                        xc[kc],
                        start=(kc == 0),
                        stop=(kc == KC - 1),
                    )
                th.append(mm_k)
            th.append(lambda: rope_evac(kT[jt], pk[0], tpos))

            def alloc_pv():
                pv[0] = pstile(ps_proj, [128, 4, HD], F32, f"pv_{b}_{jt}")

            th.append(alloc_pv)
            for tb in range(4):
                for kc in range(KC):
                    def mm_v(tb=tb, kc=kc):
                        nc.tensor.matmul(
                            pv[0][:, tb, :],
                            xc[kc][:, 128 * tb : 128 * tb + 128],
                            wv_sb[:, kc, :],
                            start=(kc == 0),
                            stop=(kc == KC - 1),
                        )
                    th.append(mm_v)
            th.append(lambda: nc.scalar.copy(state[b]["v"][jt], pv[0]))
            return th

        def oproj_thunks(b, j):
            """Thunk list for o_proj of row tile j (deferred one phase so the
            reciprocal broadcast DMA has landed)."""
            yT = state[b]["yT"]
            rbcs = state[b]["rbc"]
            th = []
            for h in range(2):
                def ymul(h=h):
                    if rbcs[h][j] is not None:
                        nc.vector.tensor_mul(yT[h][j], yT[h][j], rbcs[h][j])
                th.append(ymul)
            for t4 in range(4):
                row = b * T + 512 * j + 128 * t4
                osb = [None]

                def alloc_osb(t4=t4, osb=osb):
                    osb[0] = outpool.tile(
                        [128, C], BF16, tag="osb", name=f"osb_{b}_{j}_{t4}"
                    )

                th.append(alloc_osb)
                for n in range(C // TT):
                    op = [None]

                    def mm_o(t4=t4, n=n, op=op, osb=osb):
                        op[0] = pstile(ps_op, [128, TT], F32, f"op_{b}_{j}_{t4}_{n}")
                        nc.tensor.matmul(
                            op[0],
                            yT[0][j][:, 128 * t4 : 128 * t4 + 128],
                            wo_sb[:, 0, TT * n : TT * n + TT],
                            start=True,
                            stop=False,
                        )
                        nc.tensor.matmul(
                            op[0],
                            yT[1][j][:, 128 * t4 : 128 * t4 + 128],
                            wo_sb[:, 1, TT * n : TT * n + TT],
                            start=False,
                            stop=True,
                        )

                    th.append(mm_o)

                    def evac_o(n=n, op=op, osb=osb):
                        osl = osb[0][:, TT * n : TT * n + TT]
                        if n < 2:
                            nc.scalar.copy(osl, op[0])
                        else:
                            nc.vector.tensor_copy(osl, op[0])

                    th.append(evac_o)

                def dma_o(row=row, osb=osb):
                    nc.scalar.dma_start(
                        out=out_ap[row : row + 128, :], in_=osb[0]
                    )

                th.append(dma_o)
            return th

        bg = deque()

        def pump(n):
            for _ in range(n):
                if not bg:
                    return
                bg.popleft()()

        def drain():
            while bg:
                bg.popleft()()

        def attention(b, j):
            qT, kT, v_sb, yT, rbcs = (
                state[b]["qT"],
                state[b]["kT"],
                state[b]["v"],
                state[b]["yT"],
                state[b]["rbc"],
            )
            if j == 0:
                chunks = [(m, 128 * m) for m in (0, 1, 2, 3)]
            else:
                chunks = [(0, 0)]
                chunks += [(4 * j + m, 128 * m) for m in (0, 1, 2, 3)]
                chunks += [(c, 0) for c in range(1, 4 * j)]
            nch = len(chunks)
            # thunks to pump per chunk so bg drains over this (h, j) phase
            per = max(1, (len(bg) + 2 * nch - 1) // (2 * nch))
            for h in range(2):
                yp = pstile(ps_y, [128, TT], F32, f"yp_{b}_{h}_{j}")
                acc = (
                    accpool.tile(
                        [128, TT], F32R, tag="acc", name=f"acc_{b}_{h}_{j}"
                    )
                    if POOL_ROWSUM
                    else None
                )
                sT = [None] * nch
                pT = [None] * nch

                def emit_scores(i):
                    cch, off = chunks[i]
                    sT[i] = pstile(ps_s, [128, TT - off], F32, f"sT_{b}_{h}_{j}_{i}")
                    nc.tensor.matmul(
                        sT[i],
                        kT[cch // 4][:, 128 * (cch % 4) : 128 * (cch % 4) + 128],
                        qT[h][j][:, off:],
                        start=True,
                        stop=True,
                    )

                if not POOL_ROWSUM:
                    rp = pstile(ps_r, [1, TT], F32, f"rp_{b}_{h}_{j}")
                emit_scores(0)
                for i in range(nch):
                    cch, off = chunks[i]
                    pT[i] = ppool.tile(
                        [128, TT - off], BF16, tag="p", name=f"pT_{b}_{h}_{j}_{i}"
                    )
                    nc.scalar.activation(
                        out=pT[i],
                        in_=sT[i],
                        func=mybir.ActivationFunctionType.Exp,
                        scale=SCALE,
                    )
                    if cch >= 4 * j:  # diagonal block: causal triangle
                        nc.vector.tensor_mul(pT[i][:, 0:128], pT[i][:, 0:128], tri_sb)
                    if i + 1 < nch:
                        emit_scores(i + 1)
                    # hold bg during the last chunks so the upcoming
                    # reciprocal sits at the DVE queue head (the rp psum
                    # bank frees only after it runs)
                    if i < nch - 2:
                        pump(per)
                    nc.tensor.matmul(
                        yp[:, off:],
                        v_sb[cch // 4][:, cch % 4, :],
                        pT[i],
                        start=(i == 0),
                        stop=(i == nch - 1),
                    )
                    if POOL_ROWSUM:
                        # Pool-engine softmax-denominator accumulation
                        if i == 0:
                            nc.gpsimd.tensor_copy(acc, pT[i])
                        else:
                            nc.gpsimd.tensor_add(acc[:, off:], acc[:, off:], pT[i])
                    else:
                        nc.tensor.matmul(
                            rp[:, off:],
                            onesb_sb,
                            pT[i],
                            start=(i == 0),
                            stop=(i == nch - 1),
                        )
                if POOL_ROWSUM:
                    # one rowsum matmul per (h, j) from the accumulated P
                    rp = pstile(ps_r, [1, TT], F32, f"rp_{b}_{h}_{j}")
                    nc.tensor.matmul(rp, ones_sb, acc, start=True, stop=True)
                if b == B - 1 and j == NT - 1:
                    # last tile: nothing left to hide the DMA round trip
                    # behind, so broadcast the reciprocal on the PE and
                    # normalize inline (dtype-clean: fp32 ins, bf16 out)
                    rcp = rcpool.tile(
                        [1, TT], F32R, tag="rcpf", name=f"rcpf_{b}_{h}_{j}"
                    )
                    with nc.allow_low_precision(reason="f32r is fp32 bits"):
                        nc.vector.reciprocal(rcp, rp)
                    rbc_ps = pstile(ps_op, [128, TT], F32, f"rbp_{b}_{h}_{j}")
                    nc.tensor.matmul(rbc_ps, onesr_sb, rcp, start=True, stop=True)
                    rbc_f = rbcpool.tile(
                        [128, TT], F32, tag="rbcf", name=f"rbcf_{b}_{h}_{j}"
                    )
                    nc.scalar.copy(rbc_f, rbc_ps)
                    nc.vector.tensor_mul(yT[h][j], yp, rbc_f)
                    rbcs[h][j] = None  # oproj skips ymul
                else:
                    rcp = rcpool.tile(
                        [1, TT], BF16, tag="rcp", name=f"rcp_{b}_{h}_{j}"
                    )
                    with nc.allow_low_precision(reason="softmax denom bf16"):
                        nc.vector.reciprocal(rcp, rp)
                    nc.scalar.copy(yT[h][j], yp)  # frees the PV bank
                    rdr = drampool.tile(
                        [1, TT], BF16, tag="rdr", name=f"rdr_{b}_{h}_{j}"
                    )
                    nc.sync.dma_start(out=rdr, in_=rcp)
                    rbc = rbcpool.tile(
                        [128, TT], BF16, tag="rbc", name=f"rbc_{b}_{h}_{j}"
                    )
                    nc.sync.dma_start(
                        out=rbc,
                        in_=bass.AP(
                            tensor=rdr.tensor,
                            offset=rdr.offset,
                            ap=[[0, 128], rdr.ap[-1]],
                        ),
                    )
                    rbcs[h][j] = rbc
                pump(per)

        for b in range(B):
            state[b] = {
                "qT": [
                    [
                        qkpool.tile([128, TT], BF16, tag="qT", name=f"qT_{b}_{h}_{j}")
                        for j in range(NT)
                    ]
                    for h in range(2)
                ],
                "kT": [
                    kpool.tile([128, TT], BF16, tag="kT", name=f"kT_{b}_{j}")
                    for j in range(NT)
                ],
                "v": [
                    vpool.tile([128, 4, HD], BF16, tag="v", name=f"v_{b}_{j}")
                    for j in range(NT)
                ],
                "yT": [
                    [
                        ytpool.tile([128, TT], BF16, tag="yT", name=f"yT_{b}_{h}_{j}")
                        for j in range(NT)
                    ]
                    for h in range(2)
                ],
                "rbc": [[None] * NT for _ in range(2)],
            }

        # startup: first projection tile inline. Emit the const DMAs right
        # after the x-tile DMAs (sync queue) so cos/sin land before the
        # first rope reads them — they go on the scalar queue concurrently.
        _startup = proj_thunks(0, 0)
        _startup[0]()  # x DMAs
        load_late_consts()
        for t in _startup[1:]:
            t()
        for b in range(B):
            for j in range(NT):
                if j + 1 < NT:
                    bg.extend(proj_thunks(b, j + 1))
                elif b + 1 < B:
                    bg.extend(proj_thunks(b + 1, 0))
                if j >= 1:
                    bg.extend(oproj_thunks(b, j - 1))
                elif b >= 1:
                    bg.extend(oproj_thunks(b - 1, NT - 1))
                attention(b, j)
                drain()
        # o_proj of the final tile has nothing left to hide behind
        for t in oproj_thunks(B - 1, NT - 1):
            t()

    nc.finalize()
    return nc


_NC_CACHE = None
TRACE = False
LAST_RESULTS = None


def _get_nc():
    global _NC_CACHE
    if _NC_CACHE is None:
        _NC_CACHE = build_kernel()
    return _NC_CACHE


def kernel(x, Wq, Wk, Wv, Wo):
    x = np.asarray(x, dtype=np.float32)
    Wq = np.asarray(Wq, dtype=np.float32)
    Wk = np.asarray(Wk, dtype=np.float32)
    Wv = np.asarray(Wv, dtype=np.float32)
    Wo = np.asarray(Wo, dtype=np.float32)

    xT = np.ascontiguousarray(x.reshape(BT, C).T).astype(NP_BF16)
    sin_, cos_ = _sin_cos_np()  # [T, 128]
    cosd = np.ascontiguousarray(cos_.T)
    sinT = np.ascontiguousarray(sin_.T)
    sind = np.empty_like(sinT)
    sind[64:128] = -sinT[0:64]
    sind[0:64] = sinT[64:128]
    trid = np.triu(np.ones((128, 128), dtype=np.float32)).astype(NP_BF16)
    onesd = np.ones((128, 1), dtype=np.float32)
    onesbd = np.ones((128, 1), dtype=NP_BF16)
    onesrd = np.ones((1, 128), dtype=np.float32)

    core_ids = list(range(N_CORES))
    in_maps = []
    for c in core_ids:
        g = c // 2
        wqT = Wq[QSH * c : QSH * (c + 1)].T  # [C, 256]
        wq_h = np.ascontiguousarray(
            wqT.reshape(KC, 128, QSH).transpose(1, 0, 2)
        ).reshape(128, KC * QSH).astype(NP_BF16)
        wkT = Wk[HD * g : HD * (g + 1)].T
        wk_h = np.ascontiguousarray(
            wkT.reshape(KC, 128, HD).transpose(1, 0, 2)
        ).reshape(128, KC * HD).astype(NP_BF16)
        wvT = Wv[HD * g : HD * (g + 1)].T
        wv_h = np.ascontiguousarray(
            wvT.reshape(KC, 128, HD).transpose(1, 0, 2)
        ).reshape(128, KC * HD).astype(NP_BF16)
        woT = Wo[:, QSH * c : QSH * (c + 1)].T  # [256, C]
        wo_h = np.ascontiguousarray(
            woT.reshape(2, 128, C).transpose(1, 0, 2)
        ).reshape(128, 2 * C).astype(NP_BF16)
        in_maps.append(
            {
                "xT": xT,
                "wq": wq_h,
                "wk": wk_h,
                "wv": wv_h,
                "wo": wo_h,
                "cosd": cosd,
                "sind": sind,
                "trid": trid,
                "onesd": onesd,
                "onesbd": onesbd,
                "onesrd": onesrd,
            }
        )
    global LAST_RESULTS
    res = run_bass_kernel_spmd(_get_nc(), in_maps, core_ids, trace=TRACE)
    LAST_RESULTS = res
    total = res.results[0]["out"].astype(np.float32)
    for c in core_ids[1:]:
        total = total + res.results[c]["out"].astype(np.float32)
    return total.reshape(B, T, C)
